# revision 14
# baseline (speedup 1.0000x reference)
"""AFTSimple (attention-free transformer, simple variant) distributed Trainium2 kernel.

Reference math (B=1, S=8192, E=1024, all f32):
    Q = q @ Wq.T + bq                     # [S, E]
    K = q @ Wk.T + bk                     # [S, E]
    V = q @ Wv.T + bv                     # [S, E]
    w = softmax(K, axis=S)                # per-feature softmax over sequence
    c = sum_f sum_s w[s,f] * V[s,f]       # scalar
    Y = sigmoid(Q) * c                    # [S, E]

Distribution: shard S across 8 NeuronCores (1024 rows each).  The three
projection-weight transposes (needed because TensorE contracts over the
partition axis) are SHARDED: core i transposes a 128-row strip of each W and
the strips are exchanged with AllGathers (Wk first so the K projection can
start early, then Wv+Wq).  Per-core softmax stats (sum_s exp(K),
sum_s exp(K)*Vraw) are AllReduced (16 KiB); bv's contribution is applied
after the collective:  numer_f = AR(sum exp(K)*Vraw)_f + bv_f * AR(sum exp(K))_f.
No max-subtraction in the softmax: K values are O(1) here (|K| < ~6).

Compute dtype: bf16 matmuls with f32 PSUM accumulation; everything after the
projections is f32.
"""

import os
import sys

for _p in ("/opt/trn_rl_repo", "/root/.axon_site/_ro/trn_rl_repo"):
    if os.path.isdir(_p) and _p not in sys.path:
        sys.path.insert(0, _p)

import numpy as np

B, S, E = 1, 8192, 1024
N_CORES = 8
P = 128
S_SH = S // N_CORES          # 1024 rows of q per core
EC = E // P                  # 8 contraction chunks
FC = E // P                  # 8 output-feature chunks
SC = S_SH // P               # 8 sequence chunks per core
NHALF = 512                  # PSUM bank: 512 f32 per partition

_CACHE = {}


def _build_nc(use_collective=True):
    import concourse.bass as bass
    import concourse.bacc as bacc
    import concourse.tile as tile
    from concourse import mybir
    from concourse.masks import make_identity

    f32 = mybir.dt.float32
    bf16 = mybir.dt.bfloat16
    AF = mybir.ActivationFunctionType

    nc = bacc.Bacc("TRN2", target_bir_lowering=False, debug=False,
                   num_devices=N_CORES)

    q_ext = nc.dram_tensor("q", [S_SH, E], f32, kind="ExternalInput")
    # 128-row strips of the weight matrices (core i gets rows i*128:(i+1)*128)
    Wq_ext = nc.dram_tensor("Wq", [P, E], f32, kind="ExternalInput")
    bq_ext = nc.dram_tensor("bq", [E], f32, kind="ExternalInput")
    Wk_ext = nc.dram_tensor("Wk", [P, E], f32, kind="ExternalInput")
    bk_ext = nc.dram_tensor("bk", [E], f32, kind="ExternalInput")
    Wv_ext = nc.dram_tensor("Wv", [P, E], f32, kind="ExternalInput")
    bv_ext = nc.dram_tensor("bv", [E], f32, kind="ExternalInput")
    out_ext = nc.dram_tensor("out", [S_SH, E], f32, kind="ExternalOutput")

    # Collective bounce buffers (collectives can't touch kernel I/O tensors).
    wk_in = nc.dram_tensor("wk_in", [EC, P, P], bf16)
    wk_out = nc.dram_tensor("wk_out", [N_CORES, EC, P, P], bf16,
                            addr_space="Shared")
    wvq_in = nc.dram_tensor("wvq_in", [2, EC, P, P], bf16)
    wvq_out = nc.dram_tensor("wvq_out", [N_CORES, 2, EC, P, P], bf16,
                             addr_space="Shared")
    stats_in = nc.dram_tensor("stats_in", [P, 32], f32)
    stats_out = nc.dram_tensor("stats_out", [P, 32], f32, addr_space="Shared")
    r_dram = nc.dram_tensor("r_partial", [P], f32)
    c_dram = nc.dram_tensor("c_scalar", [1], f32)

    rg = [list(range(N_CORES))]

    from contextlib import ExitStack
    with tile.TileContext(nc) as tc, ExitStack() as ctx:
        const = ctx.enter_context(tc.tile_pool(name="const", bufs=1))
        stage = ctx.enter_context(tc.tile_pool(name="stage", bufs=5))
        persist = ctx.enter_context(tc.tile_pool(name="persist", bufs=1))
        epool = ctx.enter_context(tc.tile_pool(name="epool", bufs=3))
        small = ctx.enter_context(tc.tile_pool(name="small", bufs=1))
        ysigp = ctx.enter_context(tc.tile_pool(name="ysigp", bufs=1))
        tpsum = ctx.enter_context(tc.tile_pool(name="tpsum", bufs=2, space="PSUM"))
        kvpsum = ctx.enter_context(tc.tile_pool(name="kvpsum", bufs=4, space="PSUM"))
        qpsum = ctx.enter_context(tc.tile_pool(name="qpsum", bufs=2, space="PSUM"))

        # ---- constants -------------------------------------------------
        ident = const.tile([P, P], bf16)
        make_identity(nc, ident)
        ones1 = const.tile([1, P], bf16)
        nc.vector.memset(ones1, 1.0)

        # biases: bk/bv gathered as [128, 8] (partition p holds f = c*128+p),
        # bq as a bf16 row [1, E] for the K=1 bias matmul.
        bk_sb = const.tile([P, FC], f32)
        nc.gpsimd.dma_start(out=bk_sb, in_=bk_ext.ap().rearrange("(c p) -> p c", p=P))
        bv_sb = const.tile([P, FC], f32)
        nc.gpsimd.dma_start(out=bv_sb, in_=bv_ext.ap().rearrange("(c p) -> p c", p=P))
        bq_bf = const.tile([1, E], bf16)
        nc.gpsimd.dma_start(out=bq_bf, in_=bq_ext.ap().rearrange("(o e) -> o e", o=1))

        stats = small.tile([P, 32], f32)
        # cols: [0:8] numer h0, [8:16] numer h1, [16:24] denom h0, [24:32] denom h1

        # ---- own weight strip -> transposed bounce ---------------------
        # strip [128 f', E] -> wsb[p, e*128+f'] = WT[e*128+p, strip_base+f']
        def strip_transpose(src_dram, name):
            stg = stage.tile([P, E], bf16, tag="stg", name=f"stg_{name}")
            nc.gpsimd.dma_start(out=stg, in_=src_dram[:, :])
            wsb = stage.tile([P, E], bf16, tag="wsb", bufs=3, name=f"wsb_{name}")
            for g in range(2):
                tp = tpsum.tile([P, NHALF], bf16, tag="tp", name=f"tp_{name}{g}")
                for j in range(4):
                    e = g * 4 + j
                    nc.tensor.transpose(
                        tp[:, j * P:(j + 1) * P],
                        stg[:, e * P:(e + 1) * P],
                        ident,
                    )
                nc.any.tensor_copy(out=wsb[:, g * NHALF:(g + 1) * NHALF], in_=tp)
            return wsb

        wsb_k = strip_transpose(Wk_ext, "k")
        nc.gpsimd.dma_start(out=wk_in.ap().rearrange("e p f -> p e f"),
                            in_=wsb_k.rearrange("p (e f) -> p e f", e=EC))
        if use_collective:
            nc.gpsimd.collective_compute(
                "AllGather", mybir.AluOpType.bypass, replica_groups=rg,
                ins=[wk_in.ap().opt()], outs=[wk_out.ap().opt()])

        wsb_v = strip_transpose(Wv_ext, "v")
        nc.gpsimd.dma_start(out=wvq_in.ap()[0].rearrange("e p f -> p e f"),
                            in_=wsb_v.rearrange("p (e f) -> p e f", e=EC))
        wsb_q = strip_transpose(Wq_ext, "q")
        nc.gpsimd.dma_start(out=wvq_in.ap()[1].rearrange("e p f -> p e f"),
                            in_=wsb_q.rearrange("p (e f) -> p e f", e=EC))
        if use_collective:
            nc.gpsimd.collective_compute(
                "AllGather", mybir.AluOpType.bypass, replica_groups=rg,
                ins=[wvq_in.ap().opt()], outs=[wvq_out.ap().opt()])

        # ---- q -> qT on the PE (fills the AllGather latency window) ----
        qT = [persist.tile([P, S_SH], bf16, tag=f"qT{e}", name=f"qT{e}")
              for e in range(EC)]
        for half in range(2):
            stg = []
            for j in range(4):
                r0 = (half * 4 + j) * P
                t = stage.tile([P, E], bf16, tag="stg", name=f"qstg{half}{j}")
                nc.gpsimd.dma_start(out=t, in_=q_ext[r0:r0 + P, :])
                stg.append(t)
            for e in range(EC):
                tp = tpsum.tile([P, NHALF], bf16, tag="tp", name=f"qtp{half}{e}")
                for j in range(4):
                    nc.tensor.transpose(
                        tp[:, j * P:(j + 1) * P],
                        stg[j][:, e * P:(e + 1) * P],
                        ident,
                    )
                nc.any.tensor_copy(
                    out=qT[e][:, half * NHALF:(half + 1) * NHALF], in_=tp)

        # ---- readback of the gathered transposed weights ---------------
        def readback(src_ap, name):
            tiles = [persist.tile([P, S_SH], bf16, tag=f"{name}{e}",
                                  name=f"{name}{e}")
                     for e in range(EC)]
            for e in range(EC):
                nc.gpsimd.dma_start(
                    out=tiles[e].rearrange("p (j f) -> p j f", j=N_CORES),
                    in_=src_ap[:, e, :, :].rearrange("j p f -> p j f"))
            return tiles

        WkT = readback(wk_out.ap(), "WkT")
        WvT = readback(wvq_out.ap()[:, 0], "WvT")
        WqT = readback(wvq_out.ap()[:, 1], "WqT")

        # ---- K / V projections + softmax stats (layout [f, s]) ---------
        for f in range(FC):
            fsl = slice(f * P, (f + 1) * P)
            for h in range(2):
                hsl = slice(h * NHALF, (h + 1) * NHALF)
                kk = kvpsum.tile([P, NHALF], f32, tag="kv", name=f"kk{f}{h}")
                for e in range(EC):
                    nc.tensor.matmul(kk, lhsT=WkT[e][:, fsl], rhs=qT[e][:, hsl],
                                     start=(e == 0), stop=(e == EC - 1))
                et = epool.tile([P, NHALF], f32, tag="et", name=f"et{f}{h}")
                nc.scalar.activation(
                    out=et, in_=kk, func=AF.Exp,
                    bias=bk_sb[:, f:f + 1], scale=1.0,
                    accum_out=stats[:, 16 + h * 8 + f:17 + h * 8 + f])

                vv = kvpsum.tile([P, NHALF], f32, tag="kv", name=f"vv{f}{h}")
                for e in range(EC):
                    nc.tensor.matmul(vv, lhsT=WvT[e][:, fsl], rhs=qT[e][:, hsl],
                                     start=(e == 0), stop=(e == EC - 1))
                prod = epool.tile([P, NHALF], f32, tag="prod", name=f"prod{f}{h}")
                nc.vector.tensor_mul(prod, et, vv)
                nc.vector.reduce_sum(stats[:, h * 8 + f:1 + h * 8 + f], prod,
                                     axis=mybir.AxisListType.X)

        # ---- AllReduce of the 16 KiB stats -----------------------------
        nc.gpsimd.dma_start(out=stats_in[:, :], in_=stats)
        if use_collective:
            nc.gpsimd.collective_compute(
                "AllReduce", mybir.AluOpType.add, replica_groups=rg,
                ins=[stats_in.ap().opt()], outs=[stats_out.ap().opt()])
        else:
            nc.gpsimd.dma_start(out=stats_out[:, :], in_=stats_in[:, :])

        # ---- Q projection + sigmoid (layout [s, f]); overlaps collective
        ysig = []
        for s in range(SC):
            ssl = slice(s * P, (s + 1) * P)
            ys = ysigp.tile([P, E], f32, tag=f"ysig{s}", name=f"ysig{s}")
            for h in range(2):
                hsl = slice(h * NHALF, (h + 1) * NHALF)
                qp = qpsum.tile([P, NHALF], f32, tag="qp", name=f"qp{s}{h}")
                for e in range(EC):
                    nc.tensor.matmul(qp, lhsT=qT[e][:, ssl], rhs=WqT[e][:, hsl],
                                     start=(e == 0), stop=False)
                nc.tensor.matmul(qp, lhsT=ones1, rhs=bq_bf[:, hsl],
                                 start=False, stop=True)
                nc.scalar.activation(out=ys[:, hsl], in_=qp, func=AF.Sigmoid)
            ysig.append(ys)

        # ---- global context scalar c (off the PE) ----------------------
        statsg = small.tile([P, 32], f32)
        nc.gpsimd.dma_start(out=statsg, in_=stats_out[:, :])
        numer = small.tile([P, FC], f32)
        nc.vector.tensor_add(numer, statsg[:, 0:8], statsg[:, 8:16])
        denom = small.tile([P, FC], f32)
        nc.vector.tensor_add(denom, statsg[:, 16:24], statsg[:, 24:32])
        fix = small.tile([P, FC], f32)
        nc.vector.tensor_mul(fix, bv_sb, denom)
        nc.vector.tensor_add(numer, numer, fix)
        rec = small.tile([P, FC], f32)
        nc.vector.reciprocal(rec, denom)
        nc.vector.tensor_mul(numer, numer, rec)
        rcol = small.tile([P, 1], f32)
        nc.vector.reduce_sum(rcol, numer, axis=mybir.AxisListType.X)
        # partition reduce via DRAM roundtrip (PE is busy with Q proj)
        nc.gpsimd.dma_start(out=r_dram.ap().rearrange("(p o) -> p o", o=1),
                            in_=rcol)
        rrow = small.tile([1, P], f32)
        nc.gpsimd.dma_start(out=rrow,
                            in_=r_dram.ap().rearrange("(o p) -> o p", o=1))
        csc = small.tile([1, 1], f32)
        nc.vector.reduce_sum(csc, rrow, axis=mybir.AxisListType.X)
        nc.gpsimd.dma_start(out=c_dram.ap().rearrange("(o c) -> o c", o=1),
                            in_=csc)
        c_sb = small.tile([P, 1], f32)
        c_bcast = bass.AP(tensor=c_dram.ap().tensor, offset=0, ap=[[0, P], [1, 1]])
        nc.gpsimd.dma_start(out=c_sb, in_=c_bcast)

        # ---- Y = sigmoid(Q) * c, stream out ----------------------------
        for s in range(SC):
            nc.vector.tensor_scalar_mul(ysig[s], ysig[s], c_sb)
            nc.scalar.dma_start(out=out_ext[s * P:(s + 1) * P, :], in_=ysig[s])

    nc.compile()
    return nc


def _get_nc():
    if "nc" not in _CACHE:
        _CACHE["nc"] = _build_nc()
    return _CACHE["nc"]


def _make_in_maps(q, Wq, bq, Wk, bk, Wv, bv):
    q = np.ascontiguousarray(np.asarray(q, dtype=np.float32).reshape(S, E))
    Wq = np.ascontiguousarray(np.asarray(Wq, dtype=np.float32))
    Wk = np.ascontiguousarray(np.asarray(Wk, dtype=np.float32))
    Wv = np.ascontiguousarray(np.asarray(Wv, dtype=np.float32))
    bq = np.ascontiguousarray(np.asarray(bq, dtype=np.float32))
    bk = np.ascontiguousarray(np.asarray(bk, dtype=np.float32))
    bv = np.ascontiguousarray(np.asarray(bv, dtype=np.float32))
    in_maps = []
    for i in range(N_CORES):
        st = slice(i * P, (i + 1) * P)
        in_maps.append({
            "q": q[i * S_SH:(i + 1) * S_SH],
            "Wq": np.ascontiguousarray(Wq[st]), "bq": bq,
            "Wk": np.ascontiguousarray(Wk[st]), "bk": bk,
            "Wv": np.ascontiguousarray(Wv[st]), "bv": bv,
        })
    return in_maps


def _run(trace=False, **inputs):
    from concourse.bass_utils import run_bass_kernel_spmd
    nc = _get_nc()
    in_maps = _make_in_maps(**inputs)
    res = run_bass_kernel_spmd(nc, in_maps, core_ids=list(range(N_CORES)),
                               trace=trace)
    shards = [np.asarray(res.results[i]["out"]) for i in range(N_CORES)]
    out = np.concatenate(shards, axis=0).reshape(B, S, E).astype(np.float32)
    return out, res


def kernel(**inputs):
    out, _ = _run(trace=False, **inputs)
    return out
